# revision 7
# baseline (speedup 1.0000x reference)
"""GCN (3-layer CareerGraphGNN) on 8 Trainium2 NeuronCores.

Strategy (graph/data parallel, per sharding hint):
  - Nodes are relabeled and dealt across 8 cores x TPC tiles of 128 nodes so
    every tile carries ~equal edge count (gather load balance).
  - Per layer: local matmul (x@W1 / h@W2) -> scaled table u = dis * (hW) in
    bf16 -> AllGather the table to every core -> each core gathers the source
    rows for the edges of ITS destination nodes with dma_gather (two <=32768
    row bank windows; int16 indices; zero-row padding) -> aggregation with
    one-hot S matrices on the TensorEngine (S built by DVE iota-compare) ->
    ACT epilogue h = relu(dis*agg + b) via PSUM rank-1 bias trick.
  - Layer 3 collapses algebraically: emb = mean(A_hat(h2@W3)+b3) =
    ((c^T h2)/N)@W3 + b3 with c = A_hat^T 1 computed on the host from the
    edge structure. Each core reduces its own node shard; host sums 8
    partials and applies the tiny heads.

Self-contained: hardcodes the problem geometry, reads nothing from disk.
"""

import math
import os
import sys

STAGE = int(os.environ.get("BASSGNN_STAGE", "9"))

sys.path.insert(0, "/opt/trn_rl_repo")

import numpy as np  # noqa: E402
import ml_dtypes  # noqa: E402

from concourse import bass, bacc, mybir, tile  # noqa: E402
from concourse import bass_utils  # noqa: E402
from concourse.masks import make_identity  # noqa: E402

NCORES = 8
P = 128
GT = 4          # tiles per gather call
BANK = 32768    # dma_gather int16 index window (rows)

BF16 = mybir.dt.bfloat16
F32 = mybir.dt.float32
nbf16 = ml_dtypes.bfloat16


# ----------------------------------------------------------------------------
# host-side graph preprocessing
# ----------------------------------------------------------------------------

def _preprocess(x, edge_index):
    N, F = x.shape
    E = edge_index.shape[1]
    TPC = math.ceil(N / (NCORES * P))
    NPC = TPC * P                      # nodes per core
    Npad = NCORES * NPC
    G = NCORES * TPC                   # total tiles

    src = edge_index[0].astype(np.int64)
    dst = edge_index[1].astype(np.int64)

    deg = (np.bincount(dst, minlength=N) + 1).astype(np.float32)
    dis = (np.float32(1.0) / np.sqrt(deg)).astype(np.float32)          # [N]
    # c = A_hat^T 1 : c[s] = dis[s] * sum_{(s,d)} dis[d] + dis[s]^2
    c = dis * np.bincount(src, weights=dis[dst].astype(np.float64),
                          minlength=N).astype(np.float32) + dis * dis  # [N]

    # node relabeling: sort by in-degree, deal to tiles for edge balance
    deg_real = np.bincount(dst, minlength=Npad)
    order = np.argsort(-deg_real, kind="stable")          # sorted pos -> old id
    i = np.arange(Npad)
    g = i % G
    lane = i // G
    core = g % NCORES
    j = g // NCORES
    newid_of_sorted = core * NPC + j * P + lane
    new2old = np.empty(Npad, np.int64)
    new2old[newid_of_sorted] = order
    old2new = np.empty(Npad, np.int64)
    old2new[new2old] = np.arange(Npad)

    # edges (+ self loops) in new ids
    s_all = np.concatenate([old2new[src], old2new[np.arange(N)]])
    d_all = np.concatenate([old2new[dst], old2new[np.arange(N)]])

    SROWS = NPC + 1                   # per-rank shard: [zero row, NPC real rows]
    TROWS = NCORES * SROWS
    WOFF = max(0, TROWS - BANK)       # bank B window start row
    row = (s_all // NPC) * SROWS + 1 + (s_all % NPC)
    isB = row >= BANK                 # bank of each edge's source row

    e_core = d_all // NPC
    e_tile = (d_all % NPC) // P       # j within core
    e_dloc = d_all % P

    # position of each edge within its (core, tile, bank) bucket
    key = (e_core * TPC + e_tile) * 2 + isB
    sort_idx = np.argsort(key, kind="stable")
    ks = key[sort_idx]
    starts = np.searchsorted(ks, np.arange(G * 2))
    counts = np.diff(np.append(starts, len(ks)))
    pos_sorted = np.arange(len(ks)) - starts[ks]
    pos = np.empty(len(ks), np.int64)
    pos[sort_idx] = pos_sorted

    nA = counts[0::2].reshape(NCORES, TPC)
    nB = counts[1::2].reshape(NCORES, TPC)
    CA = int(math.ceil(nA.max() / P)) if nA.max() > 0 else 0
    CB = int(math.ceil(nB.max() / P)) if nB.max() > 0 else 0

    zrowB = (NCORES - 1) * SROWS - WOFF
    if CB:
        assert 0 <= zrowB < BANK, (zrowB, WOFF)
    # per-core flat slot arrays in (tile, chunk, lane) order
    idxA = np.zeros((NCORES, TPC * CA * P), np.int16)            # default zrowA=0
    idxB = np.full((NCORES, TPC * CB * P), zrowB, np.int16) if CB else None
    dstA = np.full((NCORES, TPC * CA * P), 255, np.int16)
    dstB = np.full((NCORES, TPC * CB * P), 255, np.int16) if CB else None

    eA = ~isB
    slotA = (e_tile * CA) * P + pos
    idxA[e_core[eA], slotA[eA]] = row[eA].astype(np.int16)
    dstA[e_core[eA], slotA[eA]] = e_dloc[eA].astype(np.int16)
    if CB:
        slotB = (e_tile * CB) * P + pos
        idxB[e_core[isB], slotB[isB]] = (row[isB] - WOFF).astype(np.int16)
        dstB[e_core[isB], slotB[isB]] = e_dloc[isB].astype(np.int16)

    # wrap index lists into the dma_gather [16-partition, col] layout per call
    def wrap(flat, C):
        if C == 0:
            return None
        per_tile = C * P
        segs = []
        for g0 in range(0, TPC, GT):
            gt = min(GT, TPC - g0)
            seg = flat[g0 * per_tile:(g0 + gt) * per_tile]
            segs.append(seg.reshape(-1, 16).T)
        return np.concatenate(segs, axis=1)

    idxA_w = np.stack([np.tile(wrap(idxA[k], CA), (8, 1)) for k in range(NCORES)])
    idxB_w = (np.stack([np.tile(wrap(idxB[k], CB), (8, 1)) for k in range(NCORES)])
              if CB else None)

    # dstloc as bf16 [128, TPC*C] column per chunk
    def dst_cols(d, C):
        if C == 0:
            return None
        return np.ascontiguousarray(
            d.reshape(NCORES, TPC * C, P).transpose(0, 2, 1)).astype(np.float32)

    dstA_c = dst_cols(dstA, CA)
    dstB_c = dst_cols(dstB, CB)

    # per-node vectors in new order, [128, TPC] column-per-tile layout
    def tile_cols(v_new, dtype):
        return np.ascontiguousarray(
            v_new.reshape(NCORES, TPC, P).transpose(0, 2, 1)).astype(dtype)

    dis_pad = np.zeros(Npad, np.float32)
    dis_pad[old2new[np.arange(N)]] = dis
    c_pad = np.zeros(Npad, np.float32)
    c_pad[old2new[np.arange(N)]] = c
    rdis_pad = np.zeros(Npad, np.float32)
    rdis_pad[old2new[np.arange(N)]] = np.float32(1.0) / dis

    dis_cols = tile_cols(dis_pad, np.float32)                  # [NC,128,TPC]
    c_cols = tile_cols(c_pad, nbf16)
    rdis_rows = rdis_pad.reshape(NCORES, TPC * P).astype(nbf16)  # [NC, NPC]

    # x: bf16, permuted, per-tile feature-major blocks [TPC*P, F]
    FC = F // P
    xfull = np.zeros((Npad, F), nbf16)
    xfull[:N] = x.astype(nbf16)
    xp = xfull[new2old]                                        # [Npad, F]
    xsh = (xp.reshape(NCORES, TPC, P, FC, P)                   # [NC,j,m,kc,f]
             .transpose(0, 1, 4, 3, 2)                         # [NC,j,f,kc,m]
             .reshape(NCORES, TPC * P, F))
    xsh = np.ascontiguousarray(xsh)

    return dict(N=N, F=F, E=E, TPC=TPC, NPC=NPC, Npad=Npad, TROWS=TROWS, SROWS=SROWS,
                WOFF=WOFF, CA=CA, CB=CB, zrowB=(zrowB if CB else 0),
                idxA_w=idxA_w, idxB_w=idxB_w, dstA_c=dstA_c, dstB_c=dstB_c,
                dis_cols=dis_cols, c_cols=c_cols, rdis_rows=rdis_rows,
                xsh=xsh, new2old=new2old, old2new=old2new)


# ----------------------------------------------------------------------------
# device program
# ----------------------------------------------------------------------------

def _build(meta, H):
    TPC, CA, CB = meta["TPC"], meta["CA"], meta["CB"]
    F, TROWS, WOFF, Npad = meta["F"], meta["TROWS"], meta["WOFF"], meta["Npad"]
    SROWS = meta["SROWS"]
    FC = F // P
    HC = H // P
    CT = CA + CB

    nc = bacc.Bacc("TRN2", target_bir_lowering=False, debug=False,
                   enable_asserts=False, num_devices=NCORES)

    d_x = nc.dram_tensor("xsh", [TPC * P, F], BF16, kind="ExternalInput").ap()
    d_w1 = nc.dram_tensor("w1", [P, FC * H], BF16, kind="ExternalInput").ap()
    d_w2 = nc.dram_tensor("w2", [P, HC * H], BF16, kind="ExternalInput").ap()
    d_b1 = nc.dram_tensor("b1row", [1, H], BF16, kind="ExternalInput").ap()
    d_b2 = nc.dram_tensor("b2row", [1, H], BF16, kind="ExternalInput").ap()
    d_rdis = nc.dram_tensor("rdis", [1, TPC * P], BF16, kind="ExternalInput").ap()
    d_dis = nc.dram_tensor("dis", [P, TPC], F32, kind="ExternalInput").ap()
    d_c = nc.dram_tensor("cc", [P, TPC], BF16, kind="ExternalInput").ap()
    d_iota = nc.dram_tensor("iota", [P, P], BF16, kind="ExternalInput").ap()
    d_ia = nc.dram_tensor("idxA", [P, TPC * CA * 8], mybir.dt.int16,
                          kind="ExternalInput").ap()
    if CB:
        d_ib = nc.dram_tensor("idxB", [P, TPC * CB * 8], mybir.dt.int16,
                              kind="ExternalInput").ap()
    d_da = nc.dram_tensor("dstA", [P, TPC * CA], F32, kind="ExternalInput").ap()
    if CB:
        d_db = nc.dram_tensor("dstB", [P, TPC * CB], F32, kind="ExternalInput").ap()
    d_out = nc.dram_tensor("out", [1, H], F32, kind="ExternalOutput").ap()

    with tile.TileContext(nc) as tc:
        with tc.tile_pool(name="dram", bufs=1, space="DRAM") as dram, \
             tc.tile_pool(name="const", bufs=1) as cpool, \
             tc.tile_pool(name="work", bufs=3) as wpool, \
             tc.tile_pool(name="gbuf", bufs=2) as gpool, \
             tc.tile_pool(name="spool", bufs=4) as spool, \
             tc.tile_pool(name="psum", bufs=2, space="PSUM") as ppool, \
             tc.tile_pool(name="psum1", bufs=1, space="PSUM") as ppool1:

            ushard = dram.tile([SROWS, H], BF16)
            tables = [dram.tile([TROWS, H], BF16, addr_space="Shared", name=f"table{i}")
                      for i in (1, 2)]

            # constants
            w1_sb = cpool.tile([P, FC * H], BF16)
            nc.sync.dma_start(out=w1_sb[:], in_=d_w1[:])
            w2_sb = cpool.tile([P, HC * H], BF16)
            nc.sync.dma_start(out=w2_sb[:], in_=d_w2[:])
            b1_sb = cpool.tile([1, H], BF16)
            nc.sync.dma_start(out=b1_sb[:], in_=d_b1[:])
            b2_sb = cpool.tile([1, H], BF16)
            nc.sync.dma_start(out=b2_sb[:], in_=d_b2[:])
            rdis_sb = cpool.tile([1, TPC * P], BF16)
            nc.sync.dma_start(out=rdis_sb[:], in_=d_rdis[:])
            dis_sb = cpool.tile([P, TPC], F32)
            nc.sync.dma_start(out=dis_sb[:], in_=d_dis[:])
            c_sb = cpool.tile([P, TPC], BF16)
            nc.sync.dma_start(out=c_sb[:], in_=d_c[:])
            iota_sb = cpool.tile([P, P], BF16)
            nc.sync.dma_start(out=iota_sb[:], in_=d_iota[:])
            ia_sb = cpool.tile([P, TPC * CA * 8], mybir.dt.int16)
            nc.sync.dma_start(out=ia_sb[:], in_=d_ia[:])
            da_sb = cpool.tile([P, TPC * CA], F32)
            nc.sync.dma_start(out=da_sb[:], in_=d_da[:])
            if CB:
                ib_sb = cpool.tile([P, TPC * CB * 8], mybir.dt.int16)
                nc.sync.dma_start(out=ib_sb[:], in_=d_ib[:])
                db_sb = cpool.tile([P, TPC * CB], F32)
                nc.sync.dma_start(out=db_sb[:], in_=d_db[:])
            ident = cpool.tile([P, P], BF16)
            make_identity(nc, ident[:])

            # zero row at the front of this rank's shard
            zt = cpool.tile([P, H], BF16)
            nc.gpsimd.memset(zt[:], 0)
            nc.sync.dma_start(out=ushard[0:1, :], in_=zt[:1, :])

            # ---- layer 1 local matmul: u1 = dis * (x @ W1) ----
            for j in range(TPC):
                xt = wpool.tile([P, F], BF16, tag="xt")
                nc.sync.dma_start(out=xt[:], in_=d_x[j * P:(j + 1) * P, :])
                ps = ppool.tile([P, H], F32, space="PSUM", tag="psmm")
                for kc in range(FC):
                    nc.tensor.matmul(out=ps[:],
                                     lhsT=xt[:, kc * P:(kc + 1) * P],
                                     rhs=w1_sb[:, kc * H:(kc + 1) * H],
                                     start=(kc == 0), stop=(kc == FC - 1))
                u = wpool.tile([P, H], BF16, tag="u")
                nc.scalar.activation(out=u[:], in_=ps[:],
                                     func=mybir.ActivationFunctionType.Copy,
                                     scale=dis_sb[:, j:j + 1])
                nc.sync.dma_start(out=ushard[1 + j * P:1 + (j + 1) * P, :], in_=u[:])

            psum3 = ppool1.tile([1, H], F32, space="PSUM")

            # ---- two aggregate layers ----
            for layer in ((1, 2) if STAGE >= 9 else (1,)):
                table = tables[layer - 1]
                if STAGE < 1:
                    break
                nc.gpsimd.collective_compute(
                    "AllGather", mybir.AluOpType.bypass,
                    replica_groups=[list(range(NCORES))],
                    ins=[ushard[:].opt()],
                    outs=[table[:].opt()],
                )
                for g0 in (range(0, TPC, GT) if STAGE >= 2 else []):
                    gt = min(GT, TPC - g0)
                    ga = gpool.tile([P, GT * CA * H], BF16, tag="ga")
                    nc.gpsimd.dma_gather(
                        out_ap=ga[:].rearrange("p (c d) -> p c d", d=H)[:, :gt * CA, :],
                        in_ap=table[0:min(TROWS, BANK), :],
                        idxs_ap=ia_sb[:, g0 * CA * 8:(g0 + gt) * CA * 8],
                        num_idxs=gt * CA * P,
                        num_idxs_reg=gt * CA * P,
                        elem_size=H, single_packet=False)
                    if CB:
                        gb = gpool.tile([P, GT * CB * H], BF16, tag="gb")
                        nc.gpsimd.dma_gather(
                            out_ap=gb[:].rearrange("p (c d) -> p c d", d=H)[:, :gt * CB, :],
                            in_ap=table[WOFF:TROWS, :],
                            idxs_ap=ib_sb[:, g0 * CB * 8:(g0 + gt) * CB * 8],
                            num_idxs=gt * CB * P,
                            num_idxs_reg=gt * CB * P,
                            elem_size=H, single_packet=False)
                    for jj in (range(gt) if STAGE >= 3 else []):
                        j = g0 + jj
                        pagg = ppool.tile([P, H], F32, space="PSUM", tag="pagg")
                        # rank-1 bias seed: psum = (1/dis) x b
                        nc.tensor.matmul(out=pagg[:],
                                         lhsT=rdis_sb[:, j * P:(j + 1) * P],
                                         rhs=(b1_sb[:] if layer == 1 else b2_sb[:]),
                                         start=True, stop=False)
                        for q in range(CT):
                            if q < CA:
                                buf = ga
                                col = jj * CA + q
                                dcol = da_sb[:, j * CA + q:j * CA + q + 1]
                            else:
                                buf = gb
                                col = jj * CB + (q - CA)
                                dcol = db_sb[:, j * CB + (q - CA):j * CB + (q - CA) + 1]
                            s = spool.tile([P, P], BF16, tag="sel")
                            nc.vector.tensor_scalar(
                                out=s[:], in0=iota_sb[:], scalar1=dcol,
                                scalar2=None, op0=mybir.AluOpType.is_equal)
                            nc.tensor.matmul(out=pagg[:], lhsT=s[:],
                                             rhs=buf[:, col * H:(col + 1) * H],
                                             start=False, stop=(q == CT - 1))
                        h = wpool.tile([P, H], BF16, tag="h")
                        nc.scalar.activation(out=h[:], in_=pagg[:],
                                             func=mybir.ActivationFunctionType.Relu,
                                             scale=dis_sb[:, j:j + 1])
                        if layer == 1:
                            # h1^T for the layer-2 matmul
                            ht = wpool.tile([P, H], BF16, tag="ht")
                            for hc in range(HC):
                                pst = ppool.tile([P, P], BF16, space="PSUM", tag="pst")
                                nc.tensor.transpose(
                                    out=pst[:], in_=h[:, hc * P:(hc + 1) * P],
                                    identity=ident[:])
                                nc.vector.tensor_copy(
                                    out=ht[:, hc * P:(hc + 1) * P], in_=pst[:])
                            ps2 = ppool.tile([P, H], F32, space="PSUM", tag="psmm")
                            for hc in range(HC):
                                nc.tensor.matmul(out=ps2[:],
                                                 lhsT=ht[:, hc * P:(hc + 1) * P],
                                                 rhs=w2_sb[:, hc * H:(hc + 1) * H],
                                                 start=(hc == 0), stop=(hc == HC - 1))
                            u2 = wpool.tile([P, H], BF16, tag="u")
                            nc.scalar.activation(
                                out=u2[:], in_=ps2[:],
                                func=mybir.ActivationFunctionType.Copy,
                                scale=dis_sb[:, j:j + 1])
                            nc.sync.dma_start(out=ushard[1 + j * P:1 + (j + 1) * P, :],
                                              in_=u2[:])
                        else:
                            # weighted column-sum: psum3 += c_j^T @ h2_j
                            nc.tensor.matmul(out=psum3[:],
                                             lhsT=c_sb[:, j:j + 1],
                                             rhs=h[:],
                                             start=(j == 0), stop=(j == TPC - 1))

            if STAGE >= 9:
                outsb = wpool.tile([1, H], F32, tag="outsb")
                nc.scalar.activation(out=outsb[:], in_=psum3[:],
                                     func=mybir.ActivationFunctionType.Copy)
                nc.sync.dma_start(out=d_out[:], in_=outsb[:])
            else:
                outsb = wpool.tile([1, H], F32, tag="outsb")
                nc.vector.tensor_copy(out=outsb[:], in_=zt[:1, :H])
                nc.sync.dma_start(out=d_out[:], in_=outsb[:])

    nc.compile()
    return nc


# ----------------------------------------------------------------------------
# entry point
# ----------------------------------------------------------------------------

def _run(inputs, trace=False, trace_kwargs=None):
    x = np.asarray(inputs["x"])
    edge_index = np.asarray(inputs["edge_index"])
    W1 = np.asarray(inputs["W1"])
    H = W1.shape[1]

    meta = _preprocess(x, edge_index)
    nc = _build(meta, H)

    FC = meta["F"] // P
    HC = H // P
    w1l = np.ascontiguousarray(
        np.asarray(W1, np.float32).reshape(FC, P, H).transpose(1, 0, 2)
        .reshape(P, FC * H)).astype(nbf16)
    w2l = np.ascontiguousarray(
        np.asarray(inputs["W2"], np.float32).reshape(HC, P, H).transpose(1, 0, 2)
        .reshape(P, HC * H)).astype(nbf16)
    b1row = np.asarray(inputs["b1"], np.float32).astype(nbf16)[None, :]
    b2row = np.asarray(inputs["b2"], np.float32).astype(nbf16)[None, :]
    iota = np.tile(np.arange(P, dtype=np.float32), (P, 1)).astype(nbf16)

    in_maps = []
    for k in range(NCORES):
        m = {
            "xsh": meta["xsh"][k],
            "w1": w1l, "w2": w2l, "b1row": b1row, "b2row": b2row,
            "rdis": meta["rdis_rows"][k][None, :],
            "dis": meta["dis_cols"][k],
            "cc": meta["c_cols"][k],
            "iota": iota,
            "idxA": meta["idxA_w"][k],
            "dstA": meta["dstA_c"][k],
        }
        if meta["CB"]:
            m["idxB"] = meta["idxB_w"][k]
            m["dstB"] = meta["dstB_c"][k]
        in_maps.append(m)

    res = bass_utils.run_bass_kernel_spmd(
        nc, in_maps, core_ids=list(range(NCORES)), trace=trace,
        trace_kwargs=trace_kwargs or {})

    r = np.zeros(H, np.float64)
    for k in range(NCORES):
        r += res.results[k]["out"][0].astype(np.float64)

    N = meta["N"]
    emb = (r.astype(np.float32) / np.float32(N)) @ np.asarray(inputs["W3"], np.float32)
    emb = (emb + np.asarray(inputs["b3"], np.float32)).astype(np.float32)[None, :]

    def head(Wn, bn):
        z = emb @ np.asarray(inputs[Wn], np.float32) + np.asarray(inputs[bn], np.float32)
        return (np.float32(1.0) / (np.float32(1.0) + np.exp(-z))).astype(np.float32)

    out = (emb, head("Wl", "bl"), head("Wi", "bi"), head("Wg", "bg"))
    return out, res


def kernel(**inputs):
    out, _ = _run(inputs, trace=False)
    return out
